# revision 33
# baseline (speedup 1.0000x reference)
"""Elman RNN (SEQ=1024, B=1024, IN=57, H=128, OUT=18) on 8 NeuronCores.

Strategy: data-parallel over batch (128 per core). Recurrence kept in
[H, B] layout so h_t is directly the moving operand of the next-step
h2h matmul. Per PSUM bank: i2h for STEPS_PER_BANK future steps is
precomputed with W_ih.T (+bias via ones-row) as one accumulation
start, then each step's h2h matmul accumulates onto its column slice
(start=False) and a single Tanh activation reads the slice into SBUF.
Batch is split into G independent groups per core to pipeline the
serial MM->ACT->MM chain.

All PE operands are fp16: hardware lowers fp32 matmuls to a fused
self-loading instruction whose sync struct holds only ONE semaphore
wait ("Too many sync wait commands" in codegen otherwise), while
2-byte dtypes get an explicit Ldweights+Matmult pair that splits the
waits. fp16 is also 1 cycle/row (4x faster than fp32) and keeps
~2^-11 relative precision. PSUM accumulation stays fp32.
"""

import os
import numpy as np

SEQ, B, IN, H, OUT = 1024, 1024, 57, 128, 18
NCORES = 8
BC = B // NCORES  # 128 batch per core
G = int(os.environ.get("RNN_G", "2"))  # batch groups per core
NG = BC // G
STEPS_PER_BANK = 512 // NG
NBANK = SEQ // STEPS_PER_BANK
DMA_STEPS = int(os.environ.get("RNN_DMA_STEPS", "8"))  # steps of x per DMA transfer (per group)
K_IN = IN + 1  # ones row folds b_ih + b_hh
I2H_SPLIT = int(os.environ.get("RNN_I2H_SPLIT", "4"))  # pieces per bank i2h
HBUFS = int(os.environ.get("RNN_HBUFS", "4"))
XBUFS = int(os.environ.get("RNN_XBUFS", "3"))

_CACHE = {}


def _build_nc():
    import concourse.tile as tile
    from concourse import bacc, mybir

    f32 = mybir.dt.float32
    f16 = mybir.dt.float16
    AF = mybir.ActivationFunctionType
    ALU = mybir.AluOpType
    AX = mybir.AxisListType

    nc = bacc.Bacc("TRN2", target_bir_lowering=False, debug=False)

    xt_d = nc.dram_tensor("xt", [K_IN, G * SEQ * NG], f16, kind="ExternalInput").ap()
    wih_d = nc.dram_tensor("wih", [K_IN, H], f16, kind="ExternalInput").ap()
    whh_d = nc.dram_tensor("whh", [H, H], f16, kind="ExternalInput").ap()
    who_d = nc.dram_tensor("who", [H, OUT], f16, kind="ExternalInput").ap()
    bho_d = nc.dram_tensor("bho", [1, OUT], f32, kind="ExternalInput").ap()
    ident_d = nc.dram_tensor("ident", [H, H], f16, kind="ExternalInput").ap()
    logp_d = nc.dram_tensor("logp", [BC, OUT], f32, kind="ExternalOutput").ap()
    ht_d = nc.dram_tensor("ht", [BC, H], f32, kind="ExternalOutput").ap()

    with tile.TileContext(nc) as tc:
        from contextlib import ExitStack

        with ExitStack() as ctx:
            const_pool = ctx.enter_context(tc.tile_pool(name="const", bufs=1))
            x_pools = [
                ctx.enter_context(tc.tile_pool(name=f"x{g}", bufs=XBUFS))
                for g in range(G)
            ]
            h_pools = [
                ctx.enter_context(tc.tile_pool(name=f"h{g}", bufs=HBUFS))
                for g in range(G)
            ]
            ps_pools = [
                ctx.enter_context(tc.tile_pool(name=f"ps{g}", bufs=3, space="PSUM"))
                for g in range(G)
            ]

            wih_t = const_pool.tile([K_IN, H], f16)
            nc.sync.dma_start(wih_t[:], wih_d[:])
            whh_t = const_pool.tile([H, H], f16)
            nc.sync.dma_start(whh_t[:], whh_d[:])
            who_t = const_pool.tile([H, OUT], f16)
            nc.sync.dma_start(who_t[:], who_d[:])
            bho_t = const_pool.tile([1, OUT], f32)
            nc.sync.dma_start(bho_t[:], bho_d[:])
            ident_t = const_pool.tile([H, H], f16)
            nc.sync.dma_start(ident_t[:], ident_d[:])
            ones_t = const_pool.tile([1, BC], f32)
            nc.gpsimd.memset(ones_t[:], 1.0)

            h_prev = []
            for g in range(G):
                h0 = h_pools[g].tile([H, NG], f16)
                nc.gpsimd.memset(h0[:], 0.0)
                h_prev.append(h0)

            def issue_x_dma(g, blk):
                t0 = blk * DMA_STEPS
                xt_tile = x_pools[g].tile([K_IN, DMA_STEPS * NG], f16)
                col0 = g * SEQ * NG + t0 * NG
                nc.sync.dma_start(
                    xt_tile[:], xt_d[:, col0 : col0 + DMA_STEPS * NG]
                )
                return xt_tile

            n_blk = SEQ // DMA_STEPS
            banks_per_blk = DMA_STEPS // STEPS_PER_BANK
            x_cur = [issue_x_dma(g, 0) for g in range(G)]
            x_nxt = [issue_x_dma(g, 1) if n_blk > 1 else None for g in range(G)]

            def issue_i2h(g, ps, xoff):
                # i2h + biases for the whole bank. Only the FIRST matmul
                # into the bank may use start=True (it marks the whole
                # 2KB zero region pending-zero); later pieces use
                # start=False, which overwrites pending-zero bytes.
                npiece = 512 // I2H_SPLIT
                for p in range(I2H_SPLIT):
                    cs = p * npiece
                    nc.tensor.matmul(
                        ps[:, cs : cs + npiece],
                        lhsT=wih_t[:],
                        rhs=x_cur[g][:, xoff + cs : xoff + cs + npiece],
                        start=(p == 0),
                        stop=False,
                        skip_group_check=True,
                    )

            def issue_step(g, ps, j):
                sl = ps[:, j * NG : (j + 1) * NG]
                nc.tensor.matmul(
                    sl,
                    lhsT=whh_t[:],
                    rhs=h_prev[g][:],
                    start=False,
                    stop=True,
                    skip_group_check=True,
                )
                h_new = h_pools[g].tile([H, NG], f16)
                nc.scalar.activation(h_new[:], sl, AF.Tanh)
                h_prev[g] = h_new

            for c in range(NBANK):
                blk, off = divmod(c, banks_per_blk)
                xoff = off * STEPS_PER_BANK * NG
                for g in range(G):
                    if c > 0 and off == 0:
                        x_cur[g] = x_nxt[g]
                        x_nxt[g] = (
                            issue_x_dma(g, blk + 1) if blk + 1 < n_blk else None
                        )
                    ps = ps_pools[g].tile([H, 512], f32)
                    issue_i2h(g, ps, xoff)
                    for j in range(STEPS_PER_BANK):
                        issue_step(g, ps, j)

            # ---- end phase: logits, log_softmax, hT transpose ----
            eps_pool = ctx.enter_context(tc.tile_pool(name="eps", bufs=1, space="PSUM"))
            # matmul PSUM outputs must start at partition 0, so each group
            # gets its own PSUM tile; ACT copies into the combined SBUF tile.
            lgs = const_pool.tile([BC, OUT], f32)
            for g in range(G):
                lg_g = eps_pool.tile([NG, OUT], f32)
                nc.tensor.matmul(
                    lg_g[:],
                    lhsT=h_prev[g][:],
                    rhs=who_t[:],
                    start=True,
                    stop=False,
                    skip_group_check=True,
                )
                nc.tensor.matmul(
                    lg_g[:],
                    lhsT=ones_t[0:1, 0:NG],
                    rhs=bho_t[:],
                    start=False,
                    stop=True,
                    skip_group_check=True,
                )
                nc.scalar.activation(
                    lgs[g * NG : (g + 1) * NG, :], lg_g[:], AF.Copy
                )

            negmax = const_pool.tile([BC, 1], f32)
            nc.vector.tensor_reduce(
                negmax[:], lgs[:], axis=AX.X, op=ALU.max, negate=True
            )
            e_t = const_pool.tile([BC, OUT], f32)
            s_t = const_pool.tile([BC, 1], f32)
            nc.scalar.activation(
                e_t[:], lgs[:], AF.Exp, bias=negmax[:], accum_out=s_t[:]
            )
            l_t = const_pool.tile([BC, 1], f32)
            nc.scalar.activation(l_t[:], s_t[:], AF.Ln)
            c2 = const_pool.tile([BC, 1], f32)
            nc.vector.tensor_sub(c2[:], negmax[:], l_t[:])
            outp = const_pool.tile([BC, OUT], f32)
            nc.scalar.activation(outp[:], lgs[:], AF.Identity, bias=c2[:])
            nc.sync.dma_start(logp_d[:], outp[:])

            ht_s = const_pool.tile([BC, H], f32)
            for g in range(G):
                htp_g = eps_pool.tile([NG, H], f16)
                nc.tensor.transpose(htp_g[:], h_prev[g][:], ident_t[:])
                nc.scalar.activation(
                    ht_s[g * NG : (g + 1) * NG, :], htp_g[:], AF.Copy
                )
            nc.sync.dma_start(ht_d[:], ht_s[:])

    return nc


def _get_nc():
    if "nc" not in _CACHE:
        _CACHE["nc"] = _build_nc()
    return _CACHE["nc"]


def _host_inputs(line_tensor, W_ih, b_ih, W_hh, b_hh, W_ho, b_ho):
    wih_h = np.concatenate(
        [W_ih.T, (b_ih + b_hh)[None, :]], axis=0
    ).astype(np.float16)  # [58, 128]
    whh_h = np.ascontiguousarray(W_hh.T).astype(np.float16)  # lhsT[k,m]=W_hh[m,k]
    who_h = np.ascontiguousarray(W_ho.T).astype(np.float16)  # [128, 18]
    bho_h = np.ascontiguousarray(b_ho[None, :]).astype(np.float32)
    ident_h = np.eye(H, dtype=np.float16)

    in_maps = []
    for c in range(NCORES):
        xc = line_tensor[:, c * BC : (c + 1) * BC, :]  # [SEQ, 128, 57]
        xt = xc.transpose(2, 0, 1)  # [57, SEQ, 128]
        xt = xt.reshape(IN, SEQ, G, NG).transpose(0, 2, 1, 3)  # [57, G, SEQ, NG]
        xt = np.concatenate(
            [xt, np.ones((1, G, SEQ, NG), dtype=line_tensor.dtype)], axis=0
        )
        in_maps.append(
            {
                "xt": np.ascontiguousarray(xt).reshape(K_IN, G * SEQ * NG).astype(np.float16),
                "wih": wih_h,
                "whh": whh_h,
                "who": who_h,
                "bho": bho_h,
                "ident": ident_h,
            }
        )
    return in_maps


def run(inputs, trace=False):
    from concourse.bass_utils import run_bass_kernel_spmd

    nc = _get_nc()
    if not nc.is_finalized():
        nc.finalize()
    in_maps = _host_inputs(**inputs)
    res = run_bass_kernel_spmd(
        nc, in_maps, list(range(NCORES)), trace=trace
    )
    logp = np.concatenate([r["logp"] for r in res.results], axis=0)
    ht = np.concatenate([r["ht"] for r in res.results], axis=0)
    return (logp, ht), res


def kernel(**inputs):
    out, _ = run(inputs, trace=False)
    return out
